# revision 12
# baseline (speedup 1.0000x reference)
"""Trainium2 Bass kernel for nn_Cross_Attention (B=8, N=2048, D=768).

Math (per batch b):
    key   = softmax(t, axis=-1).T  -> E[m,d]/R[m] transposed   (t in {x2, x3})
    query = softmax(t, axis=0)     -> E[m,d]/S[d]
    attn  = (x @ key^T) @ query = x @ KQ      with KQ [D, D]
    out   = f*(attn_1 @ W1^T + b1) + f*(attn_2 @ W2^T + b2) + x
          = x @ Msum + (x + f*(b1+b2))
    Msum  = f*(KQ_1 @ W1^T + KQ_2 @ W2^T)

so the [N, N] context never materializes.  The gram contraction is
asymmetric:  KQ[d, d'] = (sum_m (E[m,d]/R[m]) * E[m,d']) / S[d']
which needs only a DVE reciprocal of R (no sqrt, no ACT-table swap).
g1 carries a trailing ones column, so the gram matmul's last output
column IS the transposed column-sum S[d'] -- no separate colsum
matmuls, no transpose of 1/S: the drain reads its own PSUM tile.

All heavy matmuls run in fp8(e4m3) DoubleRow mode (2 contraction rows
per partition -> 2x PE rate).  Power-of-2 prescales keep every fp8
tensor centered in e4m3's normal range; they cancel in one final
scalar multiply.  Verified numerics (numpy, exact fp8/bf16 rounding):
fro rel err 2.6e-3 vs the 2e-2 budget.

Distribution: pure data-parallel, batch b -> core b, no collectives.

DMA budget per core: in x2/x3/xT (fp8) + W (fp8) + (x+fb) (bf16)
= 9.0 MB, out (bf16) 3.1 MB -- vs 36 MB for the f32 baseline.
"""

import numpy as np
import ml_dtypes

import concourse.bass as bass
import concourse.tile as tile
from concourse import bacc
from concourse import mybir
from concourse.bass_utils import run_bass_kernel_spmd

F32 = mybir.dt.float32
BF16 = mybir.dt.bfloat16
FP8 = mybir.dt.float8e4

NP_FP8 = ml_dtypes.float8_e4m3
NP_BF16 = ml_dtypes.bfloat16

B = 8
P = 128
D = 768
DT = D // P    # 6 feature subtiles
NT = 16        # 128-token tiles
TG = 4         # token tiles per DMA group
NG = NT // TG
GD = D + 1     # gram moving width: 768 data cols + the ones column
# moving-dim chunks: stay inside one PSUM bank (512 f32)
CHUNKS = ((0, 512), (512, 256))
GCHUNKS = ((0, 512), (512, 257))
# fp8 prescales (exact powers of two; cancelled in the output scale)
CR = 1024.0    # on E/R   (g1)
CS = 64.0      # on KQ    (kqt)
CW = 16.0      # on f*W^T (w8)
SO = 1.0 / (CS * CW)
DR = mybir.MatmulPerfMode.DoubleRow
MUL = mybir.AluOpType.mult
ADD = mybir.AluOpType.add
COPY = mybir.ActivationFunctionType.Copy
EXP = mybir.ActivationFunctionType.Exp


def build_nc():
    N = NT * P
    nc = bacc.Bacc()

    x2_d = nc.dram_tensor("x2", [N, D], FP8, kind="ExternalInput")
    x3_d = nc.dram_tensor("x3", [N, D], FP8, kind="ExternalInput")
    xt8_d = nc.dram_tensor("xt8", [D, N], FP8, kind="ExternalInput")  # x^T
    w8_d = nc.dram_tensor("w8", [2 * D, D], FP8, kind="ExternalInput")  # f*Wt^T*CW stacked
    xfb_d = nc.dram_tensor("xfb", [N, D], BF16, kind="ExternalInput")  # x + f*(b1+b2)
    out_d = nc.dram_tensor("out", [N, D], BF16, kind="ExternalOutput")

    att_g = [
        x2_d.rearrange("(g t p) d -> g p t d", p=P, t=TG),
        x3_d.rearrange("(g t p) d -> g p t d", p=P, t=TG),
    ]
    xt8_r = xt8_d.rearrange("(c p) n -> p c n", p=P)
    w8_r = w8_d.rearrange("(t c p) j -> p t c j", p=P, c=DT)
    xfb_r = xfb_d.rearrange("(h t p) d -> h p t d", p=P, t=NT // 2)
    out_t = out_d.rearrange("(t p) d -> t p d", p=P)

    with tile.TileContext(nc) as tc:
        with (
            tc.tile_pool(name="consts", bufs=1) as consts,
            tc.tile_pool(name="gbuf", bufs=2) as gbuf,
            tc.tile_pool(name="stream", bufs=3) as stream,
            tc.tile_pool(name="stats", bufs=2) as stats,
            tc.tile_pool(name="obuf", bufs=3) as obufp,
            tc.tile_pool(name="acc", bufs=3, space="PSUM") as acc,
            tc.tile_pool(name="tp", bufs=1, space="PSUM") as tpp,
        ):
            ones = consts.tile([P, 2, P], FP8)
            nc.vector.memset(ones, 1.0)
            # persistent fp8 operands
            kqt = [consts.tile([P, DT, D], FP8, name=f"kqt{t}") for t in range(2)]
            msum = consts.tile([P, DT, D], FP8)
            xt8 = consts.tile([P, DT, N], FP8)
            w8 = consts.tile([P, 2, DT, D], FP8)
            xfb = consts.tile([P, NT, D], BF16)

            # PE warmup (p-state ramp) on the memset ones tile
            wps = tpp.tile([P, P], F32, tag="tp", name="warm")
            for k in range(5):
                nc.tensor.matmul(
                    wps, ones, ones, start=True, stop=True, perf_mode=DR
                )

            def drain(t, dp, ps, srec):
                # kqt[d', d] = KQ_raw[d', d] * CS / (CR * S[d']);  S^T sits
                # in the gram accumulator's own last column.
                nc.vector.reciprocal(srec[:, dp : dp + 1], ps[:, D : D + 1])
                nc.vector.tensor_scalar(
                    out=kqt[t][:, dp, :], in0=ps[:, 0:D],
                    scalar1=srec[:, dp : dp + 1], scalar2=CS / CR,
                    op0=MUL, op1=MUL,
                )

            for t in range(2):
                g2 = gbuf.tile([P, NT, D], FP8, tag="g2", name=f"g2_{t}")
                g1 = gbuf.tile([P, NT, GD], FP8, tag="g1", name=f"g1_{t}")
                nc.gpsimd.memset(g1[:, :, D:GD], 1.0)  # S column
                rvec = stats.tile([P, NT], F32, tag="rvec")
                rvr = stats.tile([P, NT], F32, tag="rvr")
                srec = stats.tile([P, DT], F32, tag="srec", name=f"srec{t}")
                kq_ps = {}
                for g in range(NG):
                    xi = stream.tile([P, TG, D], FP8, tag="in", name=f"xi{t}_{g}")
                    nc.sync.dma_start(out=xi, in_=att_g[t][g])
                    for j in range(TG):
                        i = g * TG + j
                        # E tile + row-sums R (f32 accum of the pre-round exp)
                        nc.scalar.activation(
                            out=g2[:, i, :], in_=xi[:, j, :], func=EXP,
                            accum_out=rvec[:, i : i + 1],
                        )
                    # one batched reciprocal per group (DVE op overhead)
                    nc.vector.reciprocal(
                        rvr[:, g * TG : (g + 1) * TG],
                        rvec[:, g * TG : (g + 1) * TG],
                    )
                    for j in range(TG):
                        i = g * TG + j
                        # g1 = E * (CR/R)  (the key-side softmax, prescaled)
                        nc.vector.tensor_scalar(
                            out=g1[:, i, 0:D], in0=g2[:, i, :],
                            scalar1=rvr[:, i : i + 1], scalar2=CR,
                            op0=MUL, op1=MUL,
                        )
                    for ip in (2 * g, 2 * g + 1):
                        pr = slice(2 * ip, 2 * ip + 2)
                        # gram pass A: d' tiles 0-2 ride the exp phase
                        for dp in range(3):
                            if ip == 0:
                                kq_ps[dp] = acc.tile(
                                    [P, GD], F32, tag="acc", name=f"kq{t}_{dp}"
                                )
                            lhsT = g2[:, pr, dp * P : (dp + 1) * P]
                            for off, sz in GCHUNKS:
                                nc.tensor.matmul(
                                    kq_ps[dp][:, off : off + sz], lhsT,
                                    g1[:, pr, off : off + sz],
                                    start=(ip == 0), stop=(ip == 7),
                                    perf_mode=DR,
                                )

                # drain pass A into fp8 kqt, freeing PSUM for pass B
                for dp in range(3):
                    drain(t, dp, kq_ps[dp], srec)
                # gram pass B: d' tiles 3-5
                kq_psb = {
                    dp: acc.tile([P, GD], F32, tag="acc", name=f"kqb{t}_{dp}")
                    for dp in range(3, 6)
                }
                for ip in range(8):
                    pr = slice(2 * ip, 2 * ip + 2)
                    for dp in range(3, 6):
                        lhsT = g2[:, pr, dp * P : (dp + 1) * P]
                        for off, sz in GCHUNKS:
                            nc.tensor.matmul(
                                kq_psb[dp][:, off : off + sz], lhsT,
                                g1[:, pr, off : off + sz],
                                start=(ip == 0), stop=(ip == 7), perf_mode=DR,
                            )
                for dp in range(3, 6):
                    drain(t, dp, kq_psb[dp], srec)

            # bulk loads: issued after the x2/x3 streams on the sync queue
            nc.sync.dma_start(out=xfb[:, 0:8, :], in_=xfb_r[0])
            nc.sync.dma_start(out=w8, in_=w8_r)
            nc.sync.dma_start(out=xt8, in_=xt8_r)
            nc.sync.dma_start(out=xfb[:, 8:16, :], in_=xfb_r[1])

            # Msum[d, j] = sum_t sum_d' kqt[t][d', d] * w8[t][d', j]
            for d in range(DT):
                m_ps = acc.tile([P, D], F32, tag="acc", name=f"m{d}")
                for t in range(2):
                    for dpp in range(3):
                        lhsT = kqt[t][:, 2 * dpp : 2 * dpp + 2, d * P : (d + 1) * P]
                        for off, sz in CHUNKS:
                            nc.tensor.matmul(
                                m_ps[:, off : off + sz], lhsT,
                                w8[:, t, 2 * dpp : 2 * dpp + 2, off : off + sz],
                                start=(t == 0 and dpp == 0),
                                stop=(t == 1 and dpp == 2),
                                perf_mode=DR,
                            )
                # PSUM -> fp8 copy on the (idle) scalar engine
                nc.scalar.activation(out=msum[:, d, :], in_=m_ps, func=COPY)

            # y = x @ Msum;  out = y*SO + (x + fb); per-tile DMA so the
            # output stream drains as soon as each tile's epilogue lands
            for i in range(NT):
                ob = obufp.tile([P, D], BF16, tag="out", name=f"ob{i}", bufs=4)
                y_ps = acc.tile([P, D], F32, tag="acc", name=f"y{i}")
                for cp in range(3):
                    lhsT = xt8[:, 2 * cp : 2 * cp + 2, i * P : (i + 1) * P]
                    for off, sz in CHUNKS:
                        nc.tensor.matmul(
                            y_ps[:, off : off + sz], lhsT,
                            msum[:, 2 * cp : 2 * cp + 2, off : off + sz],
                            start=(cp == 0), stop=(cp == 2), perf_mode=DR,
                        )
                nc.vector.scalar_tensor_tensor(
                    out=ob, in0=y_ps, scalar=SO,
                    in1=xfb[:, i, :], op0=MUL, op1=ADD,
                )
                nc.gpsimd.dma_start(out=out_t[i], in_=ob)

    nc.compile()
    return nc


def prep_inputs(inputs):
    x = np.asarray(inputs["x"], dtype=np.float32)
    x2 = np.asarray(inputs["x2"], dtype=np.float32)
    x3 = np.asarray(inputs["x3"], dtype=np.float32)
    W1 = np.asarray(inputs["W1"], dtype=np.float32)
    b1 = np.asarray(inputs["b1"], dtype=np.float32)
    W2 = np.asarray(inputs["W2"], dtype=np.float32)
    b2 = np.asarray(inputs["b2"], dtype=np.float32)
    w = np.asarray(inputs["w"], dtype=np.float32)

    f = 1.0 / (1.0 + np.exp(-float(w.reshape(-1)[0])))
    w8 = np.concatenate(
        [(f * CW * W1).T, (f * CW * W2).T], axis=0
    ).astype(NP_FP8)
    fb = (f * (b1 + b2)).astype(np.float32)

    x2_8 = x2.astype(NP_FP8)
    x3_8 = x3.astype(NP_FP8)
    xfb = (x + fb[None, None, :]).astype(NP_BF16)
    return [
        {
            "x2": np.ascontiguousarray(x2_8[b]),
            "x3": np.ascontiguousarray(x3_8[b]),
            "xt8": np.ascontiguousarray(x[b].T).astype(NP_FP8),
            "w8": w8,
            "xfb": np.ascontiguousarray(xfb[b]),
        }
        for b in range(B)
    ]


_NC = None


def kernel(**inputs) -> np.ndarray:
    global _NC
    if _NC is None:
        _NC = build_nc()
    in_maps = prep_inputs(inputs)
    res = run_bass_kernel_spmd(_NC, in_maps, list(range(B)))
    return np.stack(
        [res.results[b]["out"] for b in range(B)], axis=0
    ).astype(np.float32)


# revision 15
# speedup vs baseline: 1.0189x; 1.0189x over previous
"""Trainium2 Bass kernel for nn_Cross_Attention (B=8, N=2048, D=768).

Math (per batch b):
    key   = softmax(t, axis=-1).T  -> E[m,d]/R[m] transposed   (t in {x2, x3})
    query = softmax(t, axis=0)     -> E[m,d]/S[d]
    attn  = (x @ key^T) @ query = x @ KQ      with KQ [D, D]
    out   = f*(attn_1 @ W1^T + b1) + f*(attn_2 @ W2^T + b2) + x
          = x @ Msum + (x + f*(b1+b2))
    Msum  = f*(KQ_1 @ W1^T + KQ_2 @ W2^T)

so the [N, N] context never materializes.  The gram contraction is
asymmetric:  KQ[d, d'] = (sum_m (E[m,d]/R[m]) * E[m,d']) / S[d']
which needs only a DVE reciprocal of R (no sqrt, no ACT-table swap).
g1 carries a trailing ones column, so the gram matmul's last output
column IS the transposed column-sum S[d'] -- no separate colsum
matmuls, no transpose of 1/S: the drain reads its own PSUM tile.

All heavy matmuls run in fp8(e4m3) DoubleRow mode (2 contraction rows
per partition -> 2x PE rate).  Power-of-2 prescales keep every fp8
tensor centered in e4m3's normal range; they cancel in one final
scalar multiply.  Verified numerics (numpy, exact fp8/bf16 rounding):
fro rel err 2.6e-3 vs the 2e-2 budget.

Distribution: pure data-parallel, batch b -> core b, no collectives.

DMA budget per core: in x2/x3/xT (fp8) + W (fp8) + (x+fb) (bf16)
= 9.0 MB, out (bf16) 3.1 MB -- vs 36 MB for the f32 baseline.
"""

import numpy as np
import ml_dtypes

import concourse.bass as bass
import concourse.tile as tile
from concourse import bacc
from concourse import mybir
from concourse.bass_utils import run_bass_kernel_spmd

F32 = mybir.dt.float32
BF16 = mybir.dt.bfloat16
FP8 = mybir.dt.float8e4

NP_FP8 = ml_dtypes.float8_e4m3
NP_BF16 = ml_dtypes.bfloat16

B = 8
P = 128
D = 768
DT = D // P    # 6 feature subtiles
NT = 16        # 128-token tiles
TG = 4         # token tiles per DMA group
NG = NT // TG
GD = D + 1     # gram moving width: 768 data cols + the ones column
# moving-dim chunks: stay inside one PSUM bank (512 f32)
CHUNKS = ((0, 512), (512, 256))
GCHUNKS = ((0, 512), (512, 257))
# fp8 prescales (exact powers of two; cancelled in the output scale)
CR = 1024.0    # on E/R   (g1)
CS = 64.0      # on KQ    (kqt)
CW = 16.0      # on f*W^T (w8)
SO = 1.0 / (CS * CW)
DR = mybir.MatmulPerfMode.DoubleRow
MUL = mybir.AluOpType.mult
ADD = mybir.AluOpType.add
COPY = mybir.ActivationFunctionType.Copy
EXP = mybir.ActivationFunctionType.Exp


def build_nc():
    N = NT * P
    nc = bacc.Bacc()

    x2_d = nc.dram_tensor("x2", [N, D], FP8, kind="ExternalInput")
    x3_d = nc.dram_tensor("x3", [N, D], FP8, kind="ExternalInput")
    xt8_d = nc.dram_tensor("xt8", [D, N], FP8, kind="ExternalInput")  # x^T
    w8_d = nc.dram_tensor("w8", [2 * D, D], FP8, kind="ExternalInput")  # f*Wt^T*CW stacked
    xfb_d = nc.dram_tensor("xfb", [N, D], BF16, kind="ExternalInput")  # x + f*(b1+b2)
    out_d = nc.dram_tensor("out", [N, D], BF16, kind="ExternalOutput")

    att_g = [
        x2_d.rearrange("(g t p) d -> g p t d", p=P, t=TG),
        x3_d.rearrange("(g t p) d -> g p t d", p=P, t=TG),
    ]
    xt8_r = xt8_d.rearrange("(c p) n -> p c n", p=P)
    w8_r = w8_d.rearrange("(t c p) j -> p t c j", p=P, c=DT)
    xfb_r = xfb_d.rearrange("(h t p) d -> h p t d", p=P, t=NT // 2)
    out_t = out_d.rearrange("(t p) d -> t p d", p=P)

    with tile.TileContext(nc) as tc:
        with (
            tc.tile_pool(name="consts", bufs=1) as consts,
            tc.tile_pool(name="gbuf", bufs=2) as gbuf,
            tc.tile_pool(name="stream", bufs=3) as stream,
            tc.tile_pool(name="stats", bufs=2) as stats,
            tc.tile_pool(name="obuf", bufs=3) as obufp,
            tc.tile_pool(name="acc", bufs=3, space="PSUM") as acc,
            tc.tile_pool(name="tp", bufs=1, space="PSUM") as tpp,
        ):
            ones = consts.tile([P, 2, P], FP8)
            nc.vector.memset(ones, 1.0)
            # persistent fp8 operands
            kqt = [consts.tile([P, DT, D], FP8, name=f"kqt{t}") for t in range(2)]
            msum = consts.tile([P, DT, D], FP8)
            xt8 = consts.tile([P, DT, N], FP8)
            w8 = consts.tile([P, 2, DT, D], FP8)
            xfb = consts.tile([P, NT, D], BF16)

            # PE warmup (p-state ramp) on the memset ones tile
            wps = tpp.tile([P, P], F32, tag="tp", name="warm")
            for k in range(5):
                nc.tensor.matmul(
                    wps, ones, ones, start=True, stop=True, perf_mode=DR
                )

            def drain(t, dp, ps, srec):
                # kqt[d', d] = KQ_raw[d', d] * CS / (CR * S[d']);  S^T sits
                # in the gram accumulator's own last column.
                nc.vector.reciprocal(srec[:, dp : dp + 1], ps[:, D : D + 1])
                nc.vector.tensor_scalar(
                    out=kqt[t][:, dp, :], in0=ps[:, 0:D],
                    scalar1=srec[:, dp : dp + 1], scalar2=CS / CR,
                    op0=MUL, op1=MUL,
                )

            for t in range(2):
                g2 = gbuf.tile([P, NT, D], FP8, tag="g2", name=f"g2_{t}")
                g1 = gbuf.tile([P, NT, GD], FP8, tag="g1", name=f"g1_{t}")
                nc.gpsimd.memset(g1[:, :, D:GD], 1.0)  # S column
                rvec = stats.tile([P, NT], F32, tag="rvec")
                rvr = stats.tile([P, NT], F32, tag="rvr")
                srec = stats.tile([P, DT], F32, tag="srec", name=f"srec{t}")
                kq_ps = {}
                for g in range(NG):
                    xi = stream.tile([P, TG, D], FP8, tag="in", name=f"xi{t}_{g}")
                    if g == 0:
                        # halve the first transfer so exp 0 starts sooner
                        nc.sync.dma_start(
                            out=xi[:, 0:2, :], in_=att_g[t][g][:, 0:2, :]
                        )
                        nc.sync.dma_start(
                            out=xi[:, 2:4, :], in_=att_g[t][g][:, 2:4, :]
                        )
                    else:
                        nc.sync.dma_start(out=xi, in_=att_g[t][g])
                    for j in range(TG):
                        i = g * TG + j
                        # E tile + row-sums R (f32 accum of the pre-round exp)
                        nc.scalar.activation(
                            out=g2[:, i, :], in_=xi[:, j, :], func=EXP,
                            accum_out=rvec[:, i : i + 1],
                        )
                    # one batched reciprocal per group (DVE op overhead)
                    nc.vector.reciprocal(
                        rvr[:, g * TG : (g + 1) * TG],
                        rvec[:, g * TG : (g + 1) * TG],
                    )
                    for j in range(TG):
                        i = g * TG + j
                        # g1 = E * (CR/R)  (the key-side softmax, prescaled)
                        nc.vector.tensor_scalar(
                            out=g1[:, i, 0:D], in0=g2[:, i, :],
                            scalar1=rvr[:, i : i + 1], scalar2=CR,
                            op0=MUL, op1=MUL,
                        )
                    for ip in (2 * g, 2 * g + 1):
                        pr = slice(2 * ip, 2 * ip + 2)
                        # gram pass A: d' tiles 0-2 ride the exp phase
                        for dp in range(3):
                            if ip == 0:
                                kq_ps[dp] = acc.tile(
                                    [P, GD], F32, tag="acc", name=f"kq{t}_{dp}"
                                )
                            lhsT = g2[:, pr, dp * P : (dp + 1) * P]
                            for off, sz in GCHUNKS:
                                nc.tensor.matmul(
                                    kq_ps[dp][:, off : off + sz], lhsT,
                                    g1[:, pr, off : off + sz],
                                    start=(ip == 0), stop=(ip == 7),
                                    perf_mode=DR,
                                )

                # drain pass A into fp8 kqt, freeing PSUM for pass B
                for dp in range(3):
                    drain(t, dp, kq_ps[dp], srec)
                # gram pass B: d' tiles 3-5
                kq_psb = {
                    dp: acc.tile([P, GD], F32, tag="acc", name=f"kqb{t}_{dp}")
                    for dp in range(3, 6)
                }
                for ip in range(8):
                    pr = slice(2 * ip, 2 * ip + 2)
                    for dp in range(3, 6):
                        lhsT = g2[:, pr, dp * P : (dp + 1) * P]
                        for off, sz in GCHUNKS:
                            nc.tensor.matmul(
                                kq_psb[dp][:, off : off + sz], lhsT,
                                g1[:, pr, off : off + sz],
                                start=(ip == 0), stop=(ip == 7), perf_mode=DR,
                            )
                for dp in range(3, 6):
                    drain(t, dp, kq_psb[dp], srec)

            # bulk loads: issued after the x2/x3 streams on the sync queue
            nc.sync.dma_start(out=xfb[:, 0:8, :], in_=xfb_r[0])
            nc.sync.dma_start(out=w8, in_=w8_r)
            nc.sync.dma_start(out=xt8, in_=xt8_r)
            nc.sync.dma_start(out=xfb[:, 8:16, :], in_=xfb_r[1])

            # Msum[d, j] = sum_t sum_d' kqt[t][d', d] * w8[t][d', j]
            for d in range(DT):
                m_ps = acc.tile([P, D], F32, tag="acc", name=f"m{d}")
                for t in range(2):
                    for dpp in range(3):
                        lhsT = kqt[t][:, 2 * dpp : 2 * dpp + 2, d * P : (d + 1) * P]
                        for off, sz in CHUNKS:
                            nc.tensor.matmul(
                                m_ps[:, off : off + sz], lhsT,
                                w8[:, t, 2 * dpp : 2 * dpp + 2, off : off + sz],
                                start=(t == 0 and dpp == 0),
                                stop=(t == 1 and dpp == 2),
                                perf_mode=DR,
                            )
                # PSUM -> fp8 copy on the (idle) scalar engine
                nc.scalar.activation(out=msum[:, d, :], in_=m_ps, func=COPY)

            # y = x @ Msum;  out = y*SO + (x + fb); per-tile DMA so the
            # output stream drains as soon as each tile's epilogue lands
            for i in range(NT):
                ob = obufp.tile([P, D], BF16, tag="out", name=f"ob{i}", bufs=4)
                y_ps = acc.tile([P, D], F32, tag="acc", name=f"y{i}")
                for cp in range(3):
                    lhsT = xt8[:, 2 * cp : 2 * cp + 2, i * P : (i + 1) * P]
                    for off, sz in CHUNKS:
                        nc.tensor.matmul(
                            y_ps[:, off : off + sz], lhsT,
                            msum[:, 2 * cp : 2 * cp + 2, off : off + sz],
                            start=(cp == 0), stop=(cp == 2), perf_mode=DR,
                        )
                nc.vector.scalar_tensor_tensor(
                    out=ob, in0=y_ps, scalar=SO,
                    in1=xfb[:, i, :], op0=MUL, op1=ADD,
                )
                nc.gpsimd.dma_start(out=out_t[i], in_=ob)

    nc.compile()
    return nc


def prep_inputs(inputs):
    x = np.asarray(inputs["x"], dtype=np.float32)
    x2 = np.asarray(inputs["x2"], dtype=np.float32)
    x3 = np.asarray(inputs["x3"], dtype=np.float32)
    W1 = np.asarray(inputs["W1"], dtype=np.float32)
    b1 = np.asarray(inputs["b1"], dtype=np.float32)
    W2 = np.asarray(inputs["W2"], dtype=np.float32)
    b2 = np.asarray(inputs["b2"], dtype=np.float32)
    w = np.asarray(inputs["w"], dtype=np.float32)

    f = 1.0 / (1.0 + np.exp(-float(w.reshape(-1)[0])))
    w8 = np.concatenate(
        [(f * CW * W1).T, (f * CW * W2).T], axis=0
    ).astype(NP_FP8)
    fb = (f * (b1 + b2)).astype(np.float32)

    x2_8 = x2.astype(NP_FP8)
    x3_8 = x3.astype(NP_FP8)
    xfb = (x + fb[None, None, :]).astype(NP_BF16)
    return [
        {
            "x2": np.ascontiguousarray(x2_8[b]),
            "x3": np.ascontiguousarray(x3_8[b]),
            "xt8": np.ascontiguousarray(x[b].T).astype(NP_FP8),
            "w8": w8,
            "xfb": np.ascontiguousarray(xfb[b]),
        }
        for b in range(B)
    ]


_NC = None


def kernel(**inputs) -> np.ndarray:
    global _NC
    if _NC is None:
        _NC = build_nc()
    in_maps = prep_inputs(inputs)
    res = run_bass_kernel_spmd(_NC, in_maps, list(range(B)))
    return np.stack(
        [res.results[b]["out"] for b in range(B)], axis=0
    ).astype(np.float32)


# revision 16
# speedup vs baseline: 1.0578x; 1.0382x over previous
"""Trainium2 Bass kernel for nn_Cross_Attention (B=8, N=2048, D=768).

Math (per batch b):
    key   = softmax(t, axis=-1).T  -> E[m,d]/R[m] transposed   (t in {x2, x3})
    query = softmax(t, axis=0)     -> E[m,d]/S[d]
    attn  = (x @ key^T) @ query = x @ KQ      with KQ [D, D]
    out   = f*(attn_1 @ W1^T + b1) + f*(attn_2 @ W2^T + b2) + x
          = x @ Msum + (x + f*(b1+b2))
    Msum  = f*(KQ_1 @ W1^T + KQ_2 @ W2^T)

so the [N, N] context never materializes.  The gram contraction is
asymmetric:  KQ[d, d'] = (sum_m (E[m,d]/R[m]) * E[m,d']) / S[d']
which needs only a DVE reciprocal of R (no sqrt, no ACT-table swap).
g1 carries a trailing ones column, so the gram matmul's last output
column IS the transposed column-sum S[d'] -- no separate colsum
matmuls, no transpose of 1/S: the drain reads its own PSUM tile.

All heavy matmuls run in fp8(e4m3) DoubleRow mode (2 contraction rows
per partition -> 2x PE rate).  Power-of-2 prescales keep every fp8
tensor centered in e4m3's normal range; they cancel in one final
scalar multiply.  Verified numerics (numpy, exact fp8/bf16 rounding):
fro rel err 2.6e-3 vs the 2e-2 budget.

Distribution: pure data-parallel, batch b -> core b, no collectives.

DMA budget per core: in x2/x3/xT (fp8) + W (fp8) + (x+fb) (bf16)
= 9.0 MB, out (bf16) 3.1 MB -- vs 36 MB for the f32 baseline.
"""

import numpy as np
import ml_dtypes

import concourse.bass as bass
import concourse.tile as tile
from concourse import bacc
from concourse import mybir
from concourse.bass_utils import run_bass_kernel_spmd

F32 = mybir.dt.float32
BF16 = mybir.dt.bfloat16
FP8 = mybir.dt.float8e4

NP_FP8 = ml_dtypes.float8_e4m3
NP_BF16 = ml_dtypes.bfloat16

B = 8
P = 128
D = 768
DT = D // P    # 6 feature subtiles
NT = 16        # 128-token tiles
TG = 4         # token tiles per DMA group
NG = NT // TG
GD = D + 1     # gram moving width: 768 data cols + the ones column
# moving-dim chunks: stay inside one PSUM bank (512 f32)
CHUNKS = ((0, 512), (512, 256))
GCHUNKS = ((0, 512), (512, 257))
# fp8 prescales (exact powers of two; cancelled in the output scale)
CR = 1024.0    # on E/R   (g1)
CS = 64.0      # on KQ    (kqt)
CW = 16.0      # on f*W^T (w8)
SO = 1.0 / (CS * CW)
DR = mybir.MatmulPerfMode.DoubleRow
MUL = mybir.AluOpType.mult
ADD = mybir.AluOpType.add
COPY = mybir.ActivationFunctionType.Copy
EXP = mybir.ActivationFunctionType.Exp


def build_nc():
    N = NT * P
    nc = bacc.Bacc()

    x2_d = nc.dram_tensor("x2", [N, D], FP8, kind="ExternalInput")
    x3_d = nc.dram_tensor("x3", [N, D], FP8, kind="ExternalInput")
    xt8_d = nc.dram_tensor("xt8", [D, N], FP8, kind="ExternalInput")  # x^T
    w8_d = nc.dram_tensor("w8", [2 * D, D], FP8, kind="ExternalInput")  # f*Wt^T*CW stacked
    xfb_d = nc.dram_tensor("xfb", [N, D], BF16, kind="ExternalInput")  # x + f*(b1+b2)
    out_d = nc.dram_tensor("out", [N, D], BF16, kind="ExternalOutput")

    att_g = [
        x2_d.rearrange("(g t p) d -> g p t d", p=P, t=TG),
        x3_d.rearrange("(g t p) d -> g p t d", p=P, t=TG),
    ]
    xt8_r = xt8_d.rearrange("(c p) n -> p c n", p=P)
    w8_r = w8_d.rearrange("(t c p) j -> p t c j", p=P, c=DT)
    xfb_r = xfb_d.rearrange("(h t p) d -> h p t d", p=P, t=NT // 2)
    out_t = out_d.rearrange("(t p) d -> t p d", p=P)

    with tile.TileContext(nc) as tc:
        with (
            tc.tile_pool(name="consts", bufs=1) as consts,
            tc.tile_pool(name="gbuf", bufs=2) as gbuf,
            tc.tile_pool(name="stream", bufs=3) as stream,
            tc.tile_pool(name="stats", bufs=2) as stats,
            tc.tile_pool(name="obuf", bufs=3) as obufp,
            tc.tile_pool(name="acc", bufs=3, space="PSUM") as acc,
            tc.tile_pool(name="tp", bufs=1, space="PSUM") as tpp,
        ):
            ones = consts.tile([P, 2, P], FP8)
            nc.vector.memset(ones, 1.0)
            # persistent fp8 operands
            kqt = [consts.tile([P, DT, D], FP8, name=f"kqt{t}") for t in range(2)]
            msum = consts.tile([P, DT, D], FP8)
            xt8 = consts.tile([P, DT, N], FP8)
            w8 = consts.tile([P, 2, DT, D], FP8)
            xfb = consts.tile([P, NT, D], BF16)

            # PE warmup (p-state ramp) on the memset ones tile
            wps = tpp.tile([P, P], F32, tag="tp", name="warm")
            for k in range(5):
                nc.tensor.matmul(
                    wps, ones, ones, start=True, stop=True, perf_mode=DR
                )

            def drain(t, dp, ps, srec):
                # kqt[d', d] = KQ_raw[d', d] * CS / (CR * S[d']);  S^T sits
                # in the gram accumulator's own last column.
                nc.vector.reciprocal(srec[:, dp : dp + 1], ps[:, D : D + 1])
                nc.vector.tensor_scalar(
                    out=kqt[t][:, dp, :], in0=ps[:, 0:D],
                    scalar1=srec[:, dp : dp + 1], scalar2=CS / CR,
                    op0=MUL, op1=MUL,
                )

            for t in range(2):
                g2 = gbuf.tile([P, NT, D], FP8, tag="g2", name=f"g2_{t}")
                g1 = gbuf.tile([P, NT, GD], FP8, tag="g1", name=f"g1_{t}")
                nc.gpsimd.memset(g1[:, :, D:GD], 1.0)  # S column
                rvec = stats.tile([P, NT], F32, tag="rvec")
                rvr = stats.tile([P, NT], F32, tag="rvr")
                srec = stats.tile([P, DT], F32, tag="srec", name=f"srec{t}")
                kq_ps = {}
                for g in range(NG):
                    xi = stream.tile([P, TG, D], FP8, tag="in", name=f"xi{t}_{g}")
                    if g == 0:
                        # halve the first transfer so exp 0 starts sooner
                        nc.sync.dma_start(
                            out=xi[:, 0:2, :], in_=att_g[t][g][:, 0:2, :]
                        )
                        nc.sync.dma_start(
                            out=xi[:, 2:4, :], in_=att_g[t][g][:, 2:4, :]
                        )
                    else:
                        nc.sync.dma_start(out=xi, in_=att_g[t][g])
                    for j in range(TG):
                        i = g * TG + j
                        # E tile + row-sums R (f32 accum of the pre-round exp)
                        nc.scalar.activation(
                            out=g2[:, i, :], in_=xi[:, j, :], func=EXP,
                            accum_out=rvec[:, i : i + 1],
                        )
                    # one batched reciprocal per group (DVE op overhead)
                    nc.vector.reciprocal(
                        rvr[:, g * TG : (g + 1) * TG],
                        rvec[:, g * TG : (g + 1) * TG],
                    )
                    for j in range(TG):
                        i = g * TG + j
                        # g1 = E * (CR/R)  (the key-side softmax, prescaled)
                        nc.vector.tensor_scalar(
                            out=g1[:, i, 0:D], in0=g2[:, i, :],
                            scalar1=rvr[:, i : i + 1], scalar2=CR,
                            op0=MUL, op1=MUL,
                        )
                    for ip in (2 * g, 2 * g + 1):
                        pr = slice(2 * ip, 2 * ip + 2)
                        # gram pass A: d' tiles 0-2 ride the exp phase
                        for dp in range(3):
                            if ip == 0:
                                kq_ps[dp] = acc.tile(
                                    [P, GD], F32, tag="acc", name=f"kq{t}_{dp}"
                                )
                            lhsT = g2[:, pr, dp * P : (dp + 1) * P]
                            for off, sz in GCHUNKS:
                                nc.tensor.matmul(
                                    kq_ps[dp][:, off : off + sz], lhsT,
                                    g1[:, pr, off : off + sz],
                                    start=(ip == 0), stop=(ip == 7),
                                    perf_mode=DR,
                                )

                # drain pass A into fp8 kqt, freeing PSUM for pass B
                for dp in range(3):
                    drain(t, dp, kq_ps[dp], srec)
                # gram pass B: d' tiles 3-5
                kq_psb = {
                    dp: acc.tile([P, GD], F32, tag="acc", name=f"kqb{t}_{dp}")
                    for dp in range(3, 6)
                }
                for ip in range(8):
                    pr = slice(2 * ip, 2 * ip + 2)
                    for dp in range(3, 6):
                        lhsT = g2[:, pr, dp * P : (dp + 1) * P]
                        for off, sz in GCHUNKS:
                            nc.tensor.matmul(
                                kq_psb[dp][:, off : off + sz], lhsT,
                                g1[:, pr, off : off + sz],
                                start=(ip == 0), stop=(ip == 7), perf_mode=DR,
                            )
                for dp in range(3, 6):
                    drain(t, dp, kq_psb[dp], srec)

            # bulk loads: issued after the x2/x3 streams on the sync queue
            nc.sync.dma_start(out=xfb[:, 0:8, :], in_=xfb_r[0])
            nc.sync.dma_start(out=w8, in_=w8_r)
            nc.sync.dma_start(out=xt8, in_=xt8_r)
            nc.sync.dma_start(out=xfb[:, 8:16, :], in_=xfb_r[1])

            # Msum[d, j] = sum_t sum_d' kqt[t][d', d] * w8[t][d', j]
            for d in range(DT):
                m_ps = acc.tile([P, D], F32, tag="acc", name=f"m{d}")
                for t in range(2):
                    for dpp in range(3):
                        lhsT = kqt[t][:, 2 * dpp : 2 * dpp + 2, d * P : (d + 1) * P]
                        for off, sz in CHUNKS:
                            nc.tensor.matmul(
                                m_ps[:, off : off + sz], lhsT,
                                w8[:, t, 2 * dpp : 2 * dpp + 2, off : off + sz],
                                start=(t == 0 and dpp == 0),
                                stop=(t == 1 and dpp == 2),
                                perf_mode=DR,
                            )
                # PSUM -> fp8 copy on the (idle) scalar engine
                nc.scalar.activation(out=msum[:, d, :], in_=m_ps, func=COPY)

            # y = x @ Msum;  out = y*SO + (x + fb); per-tile DMA so the
            # output stream drains as soon as each tile's epilogue lands
            for i in range(NT):
                ob = obufp.tile([P, D], BF16, tag="out", name=f"ob{i}", bufs=4)
                y_ps = acc.tile([P, D], F32, tag="acc", name=f"y{i}")
                for cp in range(3):
                    lhsT = xt8[:, 2 * cp : 2 * cp + 2, i * P : (i + 1) * P]
                    for off, sz in CHUNKS:
                        nc.tensor.matmul(
                            y_ps[:, off : off + sz], lhsT,
                            msum[:, 2 * cp : 2 * cp + 2, off : off + sz],
                            start=(cp == 0), stop=(cp == 2), perf_mode=DR,
                        )
                nc.vector.scalar_tensor_tensor(
                    out=ob, in0=y_ps, scalar=SO,
                    in1=xfb[:, i, :], op0=MUL, op1=ADD,
                )
                nc.sync.dma_start(out=out_t[i], in_=ob)

    nc.compile()
    return nc


def prep_inputs(inputs):
    x = np.asarray(inputs["x"], dtype=np.float32)
    x2 = np.asarray(inputs["x2"], dtype=np.float32)
    x3 = np.asarray(inputs["x3"], dtype=np.float32)
    W1 = np.asarray(inputs["W1"], dtype=np.float32)
    b1 = np.asarray(inputs["b1"], dtype=np.float32)
    W2 = np.asarray(inputs["W2"], dtype=np.float32)
    b2 = np.asarray(inputs["b2"], dtype=np.float32)
    w = np.asarray(inputs["w"], dtype=np.float32)

    f = 1.0 / (1.0 + np.exp(-float(w.reshape(-1)[0])))
    w8 = np.concatenate(
        [(f * CW * W1).T, (f * CW * W2).T], axis=0
    ).astype(NP_FP8)
    fb = (f * (b1 + b2)).astype(np.float32)

    x2_8 = x2.astype(NP_FP8)
    x3_8 = x3.astype(NP_FP8)
    xfb = (x + fb[None, None, :]).astype(NP_BF16)
    return [
        {
            "x2": np.ascontiguousarray(x2_8[b]),
            "x3": np.ascontiguousarray(x3_8[b]),
            "xt8": np.ascontiguousarray(x[b].T).astype(NP_FP8),
            "w8": w8,
            "xfb": np.ascontiguousarray(xfb[b]),
        }
        for b in range(B)
    ]


_NC = None


def kernel(**inputs) -> np.ndarray:
    global _NC
    if _NC is None:
        _NC = build_nc()
    in_maps = prep_inputs(inputs)
    res = run_bass_kernel_spmd(_NC, in_maps, list(range(B)))
    return np.stack(
        [res.results[b]["out"] for b in range(B)], axis=0
    ).astype(np.float32)
